# revision 10
# baseline (speedup 1.0000x reference)
"""GCN block (3 layers) on 8 trn2 NeuronCores, data-parallel over batch.

Math: each layer is X' = (adj + I) @ lrelu(X @ W).
Fold each layer's weight into the previous layer's output (A(HW) = (AH)W)
so every layer is one adjacency matmul plus an identity add:

    H0 = lrelu(X0 W0)
    layer l:  G_l = H_l W_{l+1}   (W3 := I)
              Z   = adj @ G_l + G_l
              H_{l+1} = lrelu(Z)   (no lrelu after layer 2)

Key precision/bandwidth trick: adj entries are uniform in [0, 2/N], tiny
relative to the identity term, so the adjacency product tolerates fp8.
We store  at8 = fp8_e4m3(S * adj^T)  with S=2048 (entries land in [0,1])
— 16 MB — which fits ENTIRELY in SBUF (128 KB/partition of 208), so it
is streamed from HBM exactly once (vs 3x for fp16 in the old version),
and fp8 runs the PE at 2x bf16 rate via DoubleRow perf mode (256-deep
contraction per instruction).

The identity term must not see fp8 noise: it is accumulated into the
same PSUM bank by one extra fp16 matmul with S*W_blk stationary, so
PSUM holds S*(adj@G + G) and the descale folds into the lrelu constants.

Per core: 8 samples x 16 features = 128 = partition width. Layouts:
    T-layout  [c=(b,d), m]   (128 partitions, N free)  for H
    N-layout  [m(part), mt, c]                          for G (fp8)
Layer 0 streams A^T panels (m-outer, all 8 output chunks accumulate in
parallel across all 8 PSUM banks). Layers 1-2 run chunk-major from the
resident A^T so each chunk's lrelu + next-layer G tiles overlap the
following chunk's accumulation.
"""

import numpy as np

N_FULL = 4096
D = 16
B_FULL = 64
NCORES = 8
B_CORE = B_FULL // NCORES  # 8
C = B_CORE * D  # 128 partitions
P = 128
NEG_SLOPE = 0.2
SCALE = 2048.0

_CACHE = {}


def _build_nc(n, free, use_double_row=True):
    import concourse.bass as bass
    import concourse.mybir as mybir
    import concourse.tile as tile
    from concourse import bacc

    f32 = mybir.dt.float32
    f16 = mybir.dt.float16
    f8 = mybir.dt.float8e4
    u8 = mybir.dt.uint8
    AF = mybir.ActivationFunctionType
    ALU = mybir.AluOpType
    DR = mybir.MatmulPerfMode.DoubleRow if use_double_row else None

    nt = n // P          # 32 m-tiles
    nch = n // free      # 8 output column chunks
    tpb = 2              # m-tiles per A^T panel (= DoubleRow k-pair)
    mb = n // (tpb * P)  # 16 panels

    nc = bacc.Bacc(
        "TRN2", target_bir_lowering=False, debug=False, num_devices=NCORES
    )
    xt_h = nc.dram_tensor("xt", [C, n], f16, kind="ExternalInput")
    at_h = nc.dram_tensor("at", [n, n], u8, kind="ExternalInput")
    w_h = nc.dram_tensor("wt", [7, P, P], f16, kind="ExternalInput")
    out_h = nc.dram_tensor("out", [C, n], f16, kind="ExternalOutput")

    def panel_src(i):
        return at_h[i * tpb * P:(i + 1) * tpb * P, :].rearrange(
            "(t p) n -> p t n", p=P
        )

    with tile.TileContext(nc) as tc:
        with (
            tc.tile_pool(name="const", bufs=1) as constp,
            tc.tile_pool(name="htp", bufs=2) as htp,
            tc.tile_pool(name="g8p", bufs=2) as g8p,
            tc.tile_pool(name="outp", bufs=4) as outp,
            tc.tile_pool(name="lkp", bufs=4) as lkp,
            tc.tile_pool(name="psp", bufs=8, space="PSUM") as psp,
        ):
            # weights (slots 0-3: W0,W1,W2,I for the tiny path; 4-6:
            # S*W1,S*W2,S*I for the identity path) + the whole X^T load
            # go on the Scalar engine's DMA queue so the Sync queue is
            # free to stream A^T panels from t=0 with no head-of-line
            # blocking. Panels alternate Sync/GpSimd queues.
            w_sb = constp.tile([P, 7, P], f16)
            nc.scalar.dma_start(w_sb[:], w_h[:].rearrange("w p q -> p w q"))
            xt_sb = constp.tile([C, n], f16)
            h = n // 2
            nc.scalar.dma_start(xt_sb[:, :h], xt_h[:, :h])
            nc.scalar.dma_start(xt_sb[:, h:], xt_h[:, h:])

            at_res = [
                constp.tile([P, tpb, n], u8, name=f"atc{i}") for i in range(mb)
            ]
            for i in range(mb):
                eng = nc.sync if i % 2 == 0 else nc.gpsimd
                eng.dma_start(at_res[i][:], panel_src(i))

            def at_mm(i):  # fp8 view of a resident panel
                return at_res[i].bitcast(f8)

            def lrelu(dest, ps, s):
                # dest = lrelu(ps/s): ACT computes relu((1-neg)/s * ps),
                # DVE adds neg/s * ps. Each instruction reads PSUM once.
                t = lkp.tile([P, dest.shape[-1]], f16, tag="lk", name="lk")
                nc.scalar.activation(
                    t[:], ps[:], AF.Relu, scale=(1.0 - NEG_SLOPE) / s
                )
                nc.vector.scalar_tensor_tensor(
                    dest, ps[:], NEG_SLOPE / s, t[:], ALU.mult, ALU.add
                )

            # ---- prepass: H0^T = lrelu(W0_blk.T @ X^T) (T-layout) ----
            ht_cur = htp.tile([C, n], f16, tag="ht", name="ht0")
            for ch in range(nch):
                sl = slice(ch * free, (ch + 1) * free)
                ps = psp.tile([P, free], f32, tag="ps", name="psx")
                nc.tensor.matmul(
                    ps[:], w_sb[:, 0, :], xt_sb[:, sl], start=True, stop=True
                )
                lrelu(ht_cur[:, sl], ps, 1.0)

            def make_g8(ht, w_idx, mts, g8_dst):
                # G tiles (N-layout, fp8) for m-tiles in mts: one tiny
                # matmul + one cast-copy each, copies alternate ACT/DVE.
                for k, mt in enumerate(mts):
                    msl = slice(mt * P, (mt + 1) * P)
                    psg = psp.tile([P, P], f32, tag="ps", name="psg")
                    nc.tensor.matmul(
                        psg[:], ht[:, msl], w_sb[:, w_idx, :],
                        start=True, stop=True,
                    )
                    if k % 2 == 0:
                        nc.vector.tensor_copy(g8_dst[:, mt, :], psg[:])
                    else:
                        nc.scalar.copy(g8_dst[:, mt, :], psg[:])

            # ---- layer 0: G0 fp8, then m-outer streamed big matmul ----
            g8_cur = g8p.tile([P, nt, P], f8, tag="g8", name="g80")
            make_g8(ht_cur, 1, range(nt), g8_cur)

            ps_list = [
                psp.tile([P, free], f32, tag="ps", name=f"ps0c{i}")
                for i in range(nch)
            ]
            for ncx in range(nch):
                sl = slice(ncx * free, (ncx + 1) * free)
                nc.tensor.matmul(
                    ps_list[ncx][:], w_sb[:, 4, :], ht_cur[:, sl],
                    start=True, stop=False,
                )
            for mbx in range(mb):
                for ncx in range(nch):
                    sl = slice(ncx * free, (ncx + 1) * free)
                    nc.tensor.matmul(
                        ps_list[ncx][:],
                        g8_cur[:, tpb * mbx:tpb * (mbx + 1), :],
                        at_mm(mbx)[:, :, sl],
                        perf_mode=DR,
                        start=False,
                        stop=(mbx == mb - 1),
                    )

            # layer 0 -> 1 turnaround: lrelu each chunk, then build G1
            # tiles for that chunk so PE/ACT/DVE pipeline across chunks.
            ht_nxt = htp.tile([C, n], f16, tag="ht", name="ht1")
            g8_nxt = g8p.tile([P, nt, P], f8, tag="g8", name="g81")
            tpc = nt // nch  # m-tiles per chunk
            for ncx in range(nch):
                sl = slice(ncx * free, (ncx + 1) * free)
                lrelu(ht_nxt[:, sl], ps_list[ncx], SCALE)
                make_g8(ht_nxt, 2, range(ncx * tpc, (ncx + 1) * tpc), g8_nxt)
            ht_cur, g8_cur = ht_nxt, g8_nxt

            # ---- layers 1-2: chunk-major from resident A^T ----
            for layer in (1, 2):
                last = layer == 2
                if not last:
                    ht_nxt = htp.tile([C, n], f16, tag="ht", name="ht2")
                    g8_nxt = g8p.tile([P, nt, P], f8, tag="g8", name="g82")
                ps_l = [None] * nch

                def issue(ncx, layer=layer):
                    sl = slice(ncx * free, (ncx + 1) * free)
                    ps = psp.tile([P, free], f32, tag="ps", name=f"psL{layer}")
                    nc.tensor.matmul(
                        ps[:], w_sb[:, 4 + layer, :], ht_cur[:, sl],
                        start=True, stop=False,
                    )
                    for kt in range(mb):
                        nc.tensor.matmul(
                            ps[:],
                            g8_cur[:, tpb * kt:tpb * (kt + 1), :],
                            at_mm(kt)[:, :, sl],
                            perf_mode=DR,
                            start=False,
                            stop=(kt == mb - 1),
                        )
                    return ps

                def finish(ncx, last=last, ht_nxt=ht_nxt if not last else None,
                           g8_nxt=g8_nxt if not last else None):
                    sl = slice(ncx * free, (ncx + 1) * free)
                    if last:
                        oc = outp.tile([P, free], f16, tag="oc", name="oc")
                        if ncx % 2 == 0:
                            nc.vector.tensor_scalar_mul(
                                oc[:], ps_l[ncx][:], 1.0 / SCALE
                            )
                        else:
                            nc.scalar.mul(oc[:], ps_l[ncx][:], 1.0 / SCALE)
                        nc.sync.dma_start(out_h[:, sl], oc[:])
                    else:
                        lrelu(ht_nxt[:, sl], ps_l[ncx], SCALE)
                        make_g8(
                            ht_nxt, 3, range(ncx * tpc, (ncx + 1) * tpc),
                            g8_nxt,
                        )

                # software pipeline: chunk ncx's finish work (lrelu +
                # next-layer G tiles) is issued two chunks later so the
                # ACT+DVE lrelu chain has a full chunk of PE time to
                # complete before the PE reaches the G-tile matmuls.
                lag = 2 if not last else 1
                for ncx in range(nch):
                    ps_l[ncx] = issue(ncx)
                    if ncx >= lag:
                        finish(ncx - lag)
                for ncx in range(nch - lag, nch):
                    finish(ncx)
                if not last:
                    ht_cur, g8_cur = ht_nxt, g8_nxt

    nc.compile()
    return nc


def _get_nc(n, free, use_double_row=True):
    key = (n, free, use_double_row)
    if key not in _CACHE:
        _CACHE[key] = _build_nc(n, free, use_double_row)
    return _CACHE[key]


def _block_diag(w, reps):
    d = w.shape[0]
    out = np.zeros((reps * d, reps * d), dtype=np.float32)
    for b in range(reps):
        out[b * d:(b + 1) * d, b * d:(b + 1) * d] = w
    return out


def prepare_inputs(x, adj, Identity, W0, W1, W2, n=N_FULL):
    """Host-side layout prep. Returns per-core input maps."""
    import ml_dtypes

    b_full = x.shape[0]
    b_core = b_full // NCORES
    c = b_core * D

    at8 = (
        np.ascontiguousarray(adj.T.astype(np.float32)) * SCALE
    ).astype(ml_dtypes.float8_e4m3).view(np.uint8)

    reps = c // D
    wb = [
        _block_diag(np.asarray(W, np.float32), reps) for W in (W0, W1, W2)
    ]
    eye = np.eye(c, dtype=np.float32)
    w_all = np.stack(
        [wb[0], wb[1], wb[2], eye,
         SCALE * wb[1], SCALE * wb[2], SCALE * eye]
    ).astype(np.float16)

    xf = np.asarray(x, np.float32)
    in_maps = []
    for core in range(NCORES):
        xs = xf[core * b_core:(core + 1) * b_core]      # (b_core, n, D)
        xt = np.ascontiguousarray(
            xs.transpose(0, 2, 1).reshape(c, n)
        ).astype(np.float16)
        in_maps.append({"xt": xt, "at": at8, "wt": w_all})
    return in_maps


def gather_output(results, n=N_FULL, b_full=B_FULL):
    b_core = b_full // NCORES
    out = np.empty((b_full, n, D), dtype=np.float32)
    for core in range(NCORES):
        oc = np.asarray(results[core]["out"], np.float32).reshape(b_core, D, n)
        out[core * b_core:(core + 1) * b_core] = oc.transpose(0, 2, 1)
    return out


def run(x, adj, Identity, W0, W1, W2, n=N_FULL, free=512, trace=False,
        use_double_row=True, **_ignored):
    from concourse.bass_utils import run_bass_kernel_spmd

    nc = _get_nc(n, free, use_double_row)
    in_maps = prepare_inputs(x, adj, Identity, W0, W1, W2, n)
    core_ids = list(range(NCORES))
    res = run_bass_kernel_spmd(nc, in_maps, core_ids, trace=trace)
    out = gather_output(res.results, n, x.shape[0])
    return out, res


def kernel(x, adj, Identity, W0, W1, W2):
    out, _ = run(x, adj, Identity, W0, W1, W2)
    return out


# revision 11
# speedup vs baseline: 1.1113x; 1.1113x over previous
"""GCN block (3 layers) on 8 trn2 NeuronCores, data-parallel over batch.

Math: each layer is X' = (adj + I) @ lrelu(X @ W).
Fold each layer's weight into the previous layer's output (A(HW) = (AH)W)
so every layer is one adjacency matmul plus an identity add:

    H0 = lrelu(X0 W0)
    layer l:  G_l = H_l W_{l+1}   (W3 := I)
              Z   = adj @ G_l + G_l
              H_{l+1} = lrelu(Z)   (no lrelu after layer 2)

Key precision/bandwidth trick: adj entries are uniform in [0, 2/N], tiny
relative to the identity term, so the adjacency product tolerates fp8.
We store  at8 = fp8_e4m3(S * adj^T)  with S=2048 (entries land in [0,1])
— 16 MB — which fits ENTIRELY in SBUF (128 KB/partition of 208), so it
is streamed from HBM exactly once (vs 3x for fp16 in the old version),
and fp8 runs the PE at 2x bf16 rate via DoubleRow perf mode (256-deep
contraction per instruction).

The identity term must not see fp8 noise: it is accumulated into the
same PSUM bank by one extra fp16 matmul with S*W_blk stationary, so
PSUM holds S*(adj@G + G) and the descale folds into the lrelu constants.

Per core: 8 samples x 16 features = 128 = partition width. Layouts:
    T-layout  [c=(b,d), m]   (128 partitions, N free)  for H
    N-layout  [m(part), mt, c]                          for G (fp8)
Layer 0 streams A^T panels (m-outer, all 8 output chunks accumulate in
parallel across all 8 PSUM banks). Layers 1-2 run chunk-major from the
resident A^T so each chunk's lrelu + next-layer G tiles overlap the
following chunk's accumulation.
"""

import numpy as np

N_FULL = 4096
D = 16
B_FULL = 64
NCORES = 8
B_CORE = B_FULL // NCORES  # 8
C = B_CORE * D  # 128 partitions
P = 128
NEG_SLOPE = 0.2
SCALE = 2048.0

_CACHE = {}


def _build_nc(n, free, use_double_row=True):
    import concourse.bass as bass
    import concourse.mybir as mybir
    import concourse.tile as tile
    from concourse import bacc

    f32 = mybir.dt.float32
    f16 = mybir.dt.float16
    f8 = mybir.dt.float8e4
    u8 = mybir.dt.uint8
    AF = mybir.ActivationFunctionType
    ALU = mybir.AluOpType
    DR = mybir.MatmulPerfMode.DoubleRow if use_double_row else None

    nt = n // P          # 32 m-tiles
    nch = n // free      # 8 output column chunks
    tpb = 2              # m-tiles per A^T panel (= DoubleRow k-pair)
    mb = n // (tpb * P)  # 16 panels

    nc = bacc.Bacc(
        "TRN2", target_bir_lowering=False, debug=False, num_devices=NCORES
    )
    xt_h = nc.dram_tensor("xt", [C, n], f16, kind="ExternalInput")
    at_h = nc.dram_tensor("at", [n, n], u8, kind="ExternalInput")
    w_h = nc.dram_tensor("wt", [7, P, P], f16, kind="ExternalInput")
    out_h = nc.dram_tensor("out", [C, n], f16, kind="ExternalOutput")

    def panel_src(i):
        return at_h[i * tpb * P:(i + 1) * tpb * P, :].rearrange(
            "(t p) n -> p t n", p=P
        )

    with tile.TileContext(nc) as tc:
        with (
            tc.tile_pool(name="const", bufs=1) as constp,
            tc.tile_pool(name="htp", bufs=2) as htp,
            tc.tile_pool(name="g8p", bufs=2) as g8p,
            tc.tile_pool(name="outp", bufs=4) as outp,
            tc.tile_pool(name="lkp", bufs=4) as lkp,
            tc.tile_pool(name="psp", bufs=8, space="PSUM") as psp,
        ):
            # One DMA queue (Sync), strict priority order: X^T halves
            # (prepass input) first, then weights, then the 16 A^T
            # panels back-to-back. Splitting across queues does NOT add
            # bandwidth on trn2 (the per-core HBM port is the cap) and
            # only starves the latency-critical transfers.
            # Weight slots 0-3: W0,W1,W2,I (tiny path); 4-6: S*W1,S*W2,
            # S*I (identity path).
            xt_sb = constp.tile([C, n], f16)
            h = n // 2
            nc.sync.dma_start(xt_sb[:, :h], xt_h[:, :h])
            nc.sync.dma_start(xt_sb[:, h:], xt_h[:, h:])
            w_sb = constp.tile([P, 7, P], f16)
            nc.sync.dma_start(w_sb[:], w_h[:].rearrange("w p q -> p w q"))

            at_res = [
                constp.tile([P, tpb, n], u8, name=f"atc{i}") for i in range(mb)
            ]
            for i in range(mb):
                nc.sync.dma_start(at_res[i][:], panel_src(i))

            def at_mm(i):  # fp8 view of a resident panel
                return at_res[i].bitcast(f8)

            def lrelu(dest, ps, s):
                # dest = lrelu(ps/s): ACT computes relu((1-neg)/s * ps),
                # DVE adds neg/s * ps. Each instruction reads PSUM once.
                t = lkp.tile([P, dest.shape[-1]], f16, tag="lk", name="lk")
                nc.scalar.activation(
                    t[:], ps[:], AF.Relu, scale=(1.0 - NEG_SLOPE) / s
                )
                nc.vector.scalar_tensor_tensor(
                    dest, ps[:], NEG_SLOPE / s, t[:], ALU.mult, ALU.add
                )

            # ---- prepass: H0^T = lrelu(W0_blk.T @ X^T) (T-layout) ----
            ht_cur = htp.tile([C, n], f16, tag="ht", name="ht0")
            for ch in range(nch):
                sl = slice(ch * free, (ch + 1) * free)
                ps = psp.tile([P, free], f32, tag="ps", name="psx")
                nc.tensor.matmul(
                    ps[:], w_sb[:, 0, :], xt_sb[:, sl], start=True, stop=True
                )
                lrelu(ht_cur[:, sl], ps, 1.0)

            def make_g8(ht, w_idx, mts, g8_dst):
                # G tiles (N-layout, fp8) for m-tiles in mts: one tiny
                # matmul + one cast-copy each, copies alternate ACT/DVE.
                for k, mt in enumerate(mts):
                    msl = slice(mt * P, (mt + 1) * P)
                    psg = psp.tile([P, P], f32, tag="ps", name="psg")
                    nc.tensor.matmul(
                        psg[:], ht[:, msl], w_sb[:, w_idx, :],
                        start=True, stop=True,
                    )
                    if k % 2 == 0:
                        nc.vector.tensor_copy(g8_dst[:, mt, :], psg[:])
                    else:
                        nc.scalar.copy(g8_dst[:, mt, :], psg[:])

            # ---- layer 0: G0 fp8, then m-outer streamed big matmul ----
            g8_cur = g8p.tile([P, nt, P], f8, tag="g8", name="g80")
            make_g8(ht_cur, 1, range(nt), g8_cur)

            ps_list = [
                psp.tile([P, free], f32, tag="ps", name=f"ps0c{i}")
                for i in range(nch)
            ]
            for ncx in range(nch):
                sl = slice(ncx * free, (ncx + 1) * free)
                nc.tensor.matmul(
                    ps_list[ncx][:], w_sb[:, 4, :], ht_cur[:, sl],
                    start=True, stop=False,
                )
            for mbx in range(mb):
                for ncx in range(nch):
                    sl = slice(ncx * free, (ncx + 1) * free)
                    nc.tensor.matmul(
                        ps_list[ncx][:],
                        g8_cur[:, tpb * mbx:tpb * (mbx + 1), :],
                        at_mm(mbx)[:, :, sl],
                        perf_mode=DR,
                        start=False,
                        stop=(mbx == mb - 1),
                    )

            # layer 0 -> 1 turnaround: lrelu each chunk, then build G1
            # tiles for that chunk so PE/ACT/DVE pipeline across chunks.
            ht_nxt = htp.tile([C, n], f16, tag="ht", name="ht1")
            g8_nxt = g8p.tile([P, nt, P], f8, tag="g8", name="g81")
            tpc = nt // nch  # m-tiles per chunk
            for ncx in range(nch):
                sl = slice(ncx * free, (ncx + 1) * free)
                lrelu(ht_nxt[:, sl], ps_list[ncx], SCALE)
                make_g8(ht_nxt, 2, range(ncx * tpc, (ncx + 1) * tpc), g8_nxt)
            ht_cur, g8_cur = ht_nxt, g8_nxt

            # ---- layers 1-2: chunk-major from resident A^T ----
            for layer in (1, 2):
                last = layer == 2
                if not last:
                    ht_nxt = htp.tile([C, n], f16, tag="ht", name="ht2")
                    g8_nxt = g8p.tile([P, nt, P], f8, tag="g8", name="g82")
                ps_l = [None] * nch

                def issue(ncx, layer=layer):
                    sl = slice(ncx * free, (ncx + 1) * free)
                    ps = psp.tile([P, free], f32, tag="ps", name=f"psL{layer}")
                    nc.tensor.matmul(
                        ps[:], w_sb[:, 4 + layer, :], ht_cur[:, sl],
                        start=True, stop=False,
                    )
                    for kt in range(mb):
                        nc.tensor.matmul(
                            ps[:],
                            g8_cur[:, tpb * kt:tpb * (kt + 1), :],
                            at_mm(kt)[:, :, sl],
                            perf_mode=DR,
                            start=False,
                            stop=(kt == mb - 1),
                        )
                    return ps

                def finish(ncx, last=last, ht_nxt=ht_nxt if not last else None,
                           g8_nxt=g8_nxt if not last else None):
                    sl = slice(ncx * free, (ncx + 1) * free)
                    if last:
                        oc = outp.tile([P, free], f16, tag="oc", name="oc")
                        if ncx % 2 == 0:
                            nc.vector.tensor_scalar_mul(
                                oc[:], ps_l[ncx][:], 1.0 / SCALE
                            )
                        else:
                            nc.scalar.mul(oc[:], ps_l[ncx][:], 1.0 / SCALE)
                        nc.sync.dma_start(out_h[:, sl], oc[:])
                    else:
                        lrelu(ht_nxt[:, sl], ps_l[ncx], SCALE)
                        make_g8(
                            ht_nxt, 3, range(ncx * tpc, (ncx + 1) * tpc),
                            g8_nxt,
                        )

                # software pipeline: chunk ncx's finish work (lrelu +
                # next-layer G tiles) is issued two chunks later so the
                # ACT+DVE lrelu chain has a full chunk of PE time to
                # complete before the PE reaches the G-tile matmuls.
                lag = 2 if not last else 1
                for ncx in range(nch):
                    ps_l[ncx] = issue(ncx)
                    if ncx >= lag:
                        finish(ncx - lag)
                for ncx in range(nch - lag, nch):
                    finish(ncx)
                if not last:
                    ht_cur, g8_cur = ht_nxt, g8_nxt

    nc.compile()
    return nc


def _get_nc(n, free, use_double_row=True):
    key = (n, free, use_double_row)
    if key not in _CACHE:
        _CACHE[key] = _build_nc(n, free, use_double_row)
    return _CACHE[key]


def _block_diag(w, reps):
    d = w.shape[0]
    out = np.zeros((reps * d, reps * d), dtype=np.float32)
    for b in range(reps):
        out[b * d:(b + 1) * d, b * d:(b + 1) * d] = w
    return out


def prepare_inputs(x, adj, Identity, W0, W1, W2, n=N_FULL):
    """Host-side layout prep. Returns per-core input maps."""
    import ml_dtypes

    b_full = x.shape[0]
    b_core = b_full // NCORES
    c = b_core * D

    at8 = (
        np.ascontiguousarray(adj.T.astype(np.float32)) * SCALE
    ).astype(ml_dtypes.float8_e4m3).view(np.uint8)

    reps = c // D
    wb = [
        _block_diag(np.asarray(W, np.float32), reps) for W in (W0, W1, W2)
    ]
    eye = np.eye(c, dtype=np.float32)
    w_all = np.stack(
        [wb[0], wb[1], wb[2], eye,
         SCALE * wb[1], SCALE * wb[2], SCALE * eye]
    ).astype(np.float16)

    xf = np.asarray(x, np.float32)
    in_maps = []
    for core in range(NCORES):
        xs = xf[core * b_core:(core + 1) * b_core]      # (b_core, n, D)
        xt = np.ascontiguousarray(
            xs.transpose(0, 2, 1).reshape(c, n)
        ).astype(np.float16)
        in_maps.append({"xt": xt, "at": at8, "wt": w_all})
    return in_maps


def gather_output(results, n=N_FULL, b_full=B_FULL):
    b_core = b_full // NCORES
    out = np.empty((b_full, n, D), dtype=np.float32)
    for core in range(NCORES):
        oc = np.asarray(results[core]["out"], np.float32).reshape(b_core, D, n)
        out[core * b_core:(core + 1) * b_core] = oc.transpose(0, 2, 1)
    return out


def run(x, adj, Identity, W0, W1, W2, n=N_FULL, free=512, trace=False,
        use_double_row=True, **_ignored):
    from concourse.bass_utils import run_bass_kernel_spmd

    nc = _get_nc(n, free, use_double_row)
    in_maps = prepare_inputs(x, adj, Identity, W0, W1, W2, n)
    core_ids = list(range(NCORES))
    res = run_bass_kernel_spmd(nc, in_maps, core_ids, trace=trace)
    out = gather_output(res.results, n, x.shape[0])
    return out, res


def kernel(x, adj, Identity, W0, W1, W2):
    out, _ = run(x, adj, Identity, W0, W1, W2)
    return out


# revision 21
# speedup vs baseline: 1.1281x; 1.0152x over previous
"""GCN block (3 layers) on 8 trn2 NeuronCores, data-parallel over batch.

Math: each layer is X' = (adj + I) @ lrelu(X @ W).
Fold each layer's weight into the previous layer's output (A(HW) = (AH)W)
so every layer is one adjacency matmul plus an identity add:

    H0 = lrelu(X0 W0)
    layer l:  G_l = H_l W_{l+1}   (W3 := I)
              Z   = adj @ G_l + G_l
              H_{l+1} = lrelu(Z)   (no lrelu after layer 2)

Key precision/bandwidth trick: adj entries are uniform in [0, 2/N], tiny
relative to the identity term, so the adjacency product tolerates fp8.
We store  at8 = fp8_e4m3(S * adj^T)  with S=2048 (entries land in [0,1])
— 16 MB — which fits ENTIRELY in SBUF (128 KB/partition of 208), so it
is streamed from HBM exactly once (vs 3x for fp16 in the old version),
and fp8 runs the PE at 2x bf16 rate via DoubleRow perf mode (256-deep
contraction per instruction).

The identity term must not see fp8 noise: it is accumulated into the
same PSUM bank by one extra fp16 matmul with S*W_blk stationary, so
PSUM holds S*(adj@G + G) and the descale folds into the lrelu constants.

Per core: 8 samples x 16 features = 128 = partition width. Layouts:
    T-layout  [c=(b,d), m]   (128 partitions, N free)  for H
    N-layout  [m(part), mt, c]                          for G (fp8)
Layer 0 streams A^T panels (m-outer, all 8 output chunks accumulate in
parallel across all 8 PSUM banks). Layers 1-2 run chunk-major from the
resident A^T so each chunk's lrelu + next-layer G tiles overlap the
following chunk's accumulation.
"""

import numpy as np

N_FULL = 4096
D = 16
B_FULL = 64
NCORES = 8
B_CORE = B_FULL // NCORES  # 8
C = B_CORE * D  # 128 partitions
P = 128
NEG_SLOPE = 0.2
SCALE = 2048.0

_CACHE = {}


def _build_nc(n, free, use_double_row=True, use_lrelu_act=False):
    import concourse.bass as bass
    import concourse.mybir as mybir
    import concourse.tile as tile
    from concourse import bacc

    f32 = mybir.dt.float32
    f16 = mybir.dt.float16
    f8 = mybir.dt.float8e4
    u8 = mybir.dt.uint8
    AF = mybir.ActivationFunctionType
    ALU = mybir.AluOpType
    DR = mybir.MatmulPerfMode.DoubleRow if use_double_row else None

    nt = n // P          # 32 m-tiles
    nch = n // free      # 8 output column chunks
    tpb = 2              # m-tiles per A^T panel (= DoubleRow k-pair)
    mb = n // (tpb * P)  # 16 panels

    nc = bacc.Bacc(
        "TRN2", target_bir_lowering=False, debug=False, num_devices=NCORES
    )
    xt_h = nc.dram_tensor("xt", [C, n], f16, kind="ExternalInput")
    at_h = nc.dram_tensor("at", [n, n], u8, kind="ExternalInput")
    w_h = nc.dram_tensor("wt", [7, P, P], f16, kind="ExternalInput")
    out_h = nc.dram_tensor("out", [C, n], f16, kind="ExternalOutput")

    def panel_src(i):
        return at_h[i * tpb * P:(i + 1) * tpb * P, :].rearrange(
            "(t p) n -> p t n", p=P
        )

    with tile.TileContext(nc) as tc:
        with (
            tc.tile_pool(name="const", bufs=1) as constp,
            tc.tile_pool(name="htp", bufs=2) as htp,
            tc.tile_pool(name="g8p", bufs=2) as g8p,
            tc.tile_pool(name="outp", bufs=4) as outp,
            tc.tile_pool(name="lkp", bufs=4) as lkp,
            tc.tile_pool(name="psp", bufs=8, space="PSUM") as psp,
        ):
            # One DMA queue (Sync), strict priority order: X^T halves
            # (prepass input) first, then weights, then the 16 A^T
            # panels back-to-back. Splitting across queues does NOT add
            # bandwidth on trn2 (the per-core HBM port is the cap) and
            # only starves the latency-critical transfers.
            # Weight slots 0-3: W0,W1,W2,I (tiny path); 4-6: S*W1,S*W2,
            # S*I (identity path).
            xt_sb = constp.tile([C, n], f16)
            h = n // 2
            nc.sync.dma_start(xt_sb[:, :h], xt_h[:, :h])
            nc.sync.dma_start(xt_sb[:, h:], xt_h[:, h:])
            w_sb = constp.tile([P, 7, P], f16)
            nc.sync.dma_start(w_sb[:], w_h[:].rearrange("w p q -> p w q"))

            at_res = [
                constp.tile([P, tpb, n], u8, name=f"atc{i}") for i in range(mb)
            ]
            for i in range(mb):
                nc.sync.dma_start(at_res[i][:], panel_src(i))

            def at_mm(i):  # fp8 view of a resident panel
                return at_res[i].bitcast(f8)

            def lrelu(dest, ps, s):
                # dest = lrelu(ps/s).
                if use_lrelu_act:
                    # single ACT instruction, alpha = negative slope
                    nc.scalar.activation(
                        dest, ps[:], AF.Lrelu, scale=1.0 / s, alpha=NEG_SLOPE
                    )
                    return
                # fallback: ACT computes relu((1-neg)/s * ps), DVE adds
                # neg/s * ps. Each instruction reads PSUM once.
                t = lkp.tile([P, dest.shape[-1]], f16, tag="lk", name="lk")
                nc.scalar.activation(
                    t[:], ps[:], AF.Relu, scale=(1.0 - NEG_SLOPE) / s
                )
                nc.vector.scalar_tensor_tensor(
                    dest, ps[:], NEG_SLOPE / s, t[:], ALU.mult, ALU.add
                )

            # ---- prepass: H0^T = lrelu(W0_blk.T @ X^T) (T-layout) ----
            ht_cur = htp.tile([C, n], f16, tag="ht", name="ht0")
            for ch in range(nch):
                sl = slice(ch * free, (ch + 1) * free)
                ps = psp.tile([P, free], f32, tag="ps", name="psx")
                nc.tensor.matmul(
                    ps[:], w_sb[:, 0, :], xt_sb[:, sl], start=True, stop=True
                )
                lrelu(ht_cur[:, sl], ps, 1.0)

            tpc = nt // nch  # m-tiles per output chunk

            def make_g8(ht, w_idx, ncx, g8_dst):
                # G tiles (N-layout, fp8) for chunk ncx's m-tiles: tpc
                # tiny transpose-matmuls into slices of ONE psum bank,
                # then a single wide cast-copy (alternating DVE/ACT).
                pst = psp.tile([P, tpc, P], f32, tag="ps", name="pst")
                for j in range(tpc):
                    mt = ncx * tpc + j
                    nc.tensor.matmul(
                        pst[:, j, :], ht[:, mt * P:(mt + 1) * P],
                        w_sb[:, w_idx, :], start=True, stop=True,
                    )
                dst = g8_dst[:, ncx * tpc:(ncx + 1) * tpc, :]
                if ncx % 2 == 0:
                    nc.vector.tensor_copy(dst, pst[:])
                else:
                    nc.scalar.copy(dst, pst[:])

            # ---- layer 0: G0 fp8, then m-outer streamed big matmul ----
            g8_cur = g8p.tile([P, nt, P], f8, tag="g8", name="g80")
            for ncx in range(nch):
                make_g8(ht_cur, 1, ncx, g8_cur)

            ps_list = [
                psp.tile([P, free], f32, tag="ps", name=f"ps0c{i}")
                for i in range(nch)
            ]
            for ncx in range(nch):
                sl = slice(ncx * free, (ncx + 1) * free)
                nc.tensor.matmul(
                    ps_list[ncx][:], w_sb[:, 4, :], ht_cur[:, sl],
                    start=True, stop=False,
                )
            for mbx in range(mb):
                for ncx in range(nch):
                    sl = slice(ncx * free, (ncx + 1) * free)
                    nc.tensor.matmul(
                        ps_list[ncx][:],
                        g8_cur[:, tpb * mbx:tpb * (mbx + 1), :],
                        at_mm(mbx)[:, :, sl],
                        perf_mode=DR,
                        start=False,
                        stop=(mbx == mb - 1),
                    )

            # layer 0 -> 1 turnaround: lrelu each chunk, then build G1
            # tiles for that chunk so PE/ACT/DVE pipeline across chunks.
            ht_nxt = htp.tile([C, n], f16, tag="ht", name="ht1")
            g8_nxt = g8p.tile([P, nt, P], f8, tag="g8", name="g81")
            for ncx in range(nch):
                sl = slice(ncx * free, (ncx + 1) * free)
                lrelu(ht_nxt[:, sl], ps_list[ncx], SCALE)
                make_g8(ht_nxt, 2, ncx, g8_nxt)
            ht_cur, g8_cur = ht_nxt, g8_nxt

            # ---- layers 1-2: chunk-major from resident A^T ----
            for layer in (1, 2):
                last = layer == 2
                if not last:
                    ht_nxt = htp.tile([C, n], f16, tag="ht", name="ht2")
                    g8_nxt = g8p.tile([P, nt, P], f8, tag="g8", name="g82")
                ps_l = [None] * nch

                def issue(ncx, layer=layer):
                    sl = slice(ncx * free, (ncx + 1) * free)
                    ps = psp.tile([P, free], f32, tag="ps", name=f"psL{layer}")
                    nc.tensor.matmul(
                        ps[:], w_sb[:, 4 + layer, :], ht_cur[:, sl],
                        start=True, stop=False,
                    )
                    for kt in range(mb):
                        nc.tensor.matmul(
                            ps[:],
                            g8_cur[:, tpb * kt:tpb * (kt + 1), :],
                            at_mm(kt)[:, :, sl],
                            perf_mode=DR,
                            start=False,
                            stop=(kt == mb - 1),
                        )
                    return ps

                def finish(ncx, last=last, ht_nxt=ht_nxt if not last else None,
                           g8_nxt=g8_nxt if not last else None):
                    sl = slice(ncx * free, (ncx + 1) * free)
                    if last:
                        oc = outp.tile([P, free], f16, tag="oc", name="oc")
                        if ncx % 2 == 0:
                            nc.vector.tensor_scalar_mul(
                                oc[:], ps_l[ncx][:], 1.0 / SCALE
                            )
                        else:
                            nc.scalar.mul(oc[:], ps_l[ncx][:], 1.0 / SCALE)
                        nc.sync.dma_start(out_h[:, sl], oc[:])
                    else:
                        lrelu(ht_nxt[:, sl], ps_l[ncx], SCALE)
                        make_g8(ht_nxt, 3, ncx, g8_nxt)

                # software pipeline: chunk ncx's finish work (lrelu +
                # next-layer G tiles) is issued two chunks later so the
                # ACT+DVE lrelu chain has a full chunk of PE time to
                # complete before the PE reaches the G-tile matmuls.
                lag = 2 if not last else 1
                for ncx in range(nch):
                    ps_l[ncx] = issue(ncx)
                    if ncx >= lag:
                        finish(ncx - lag)
                for ncx in range(nch - lag, nch):
                    finish(ncx)
                if not last:
                    ht_cur, g8_cur = ht_nxt, g8_nxt

    nc.compile()
    return nc


def _get_nc(n, free, use_double_row=True, use_lrelu_act=False):
    key = (n, free, use_double_row, use_lrelu_act)
    if key not in _CACHE:
        _CACHE[key] = _build_nc(n, free, use_double_row, use_lrelu_act)
    return _CACHE[key]


def _block_diag(w, reps):
    d = w.shape[0]
    out = np.zeros((reps * d, reps * d), dtype=np.float32)
    for b in range(reps):
        out[b * d:(b + 1) * d, b * d:(b + 1) * d] = w
    return out


def prepare_inputs(x, adj, Identity, W0, W1, W2, n=N_FULL):
    """Host-side layout prep. Returns per-core input maps."""
    import ml_dtypes

    b_full = x.shape[0]
    b_core = b_full // NCORES
    c = b_core * D

    at8 = (
        np.ascontiguousarray(adj.T.astype(np.float32)) * SCALE
    ).astype(ml_dtypes.float8_e4m3).view(np.uint8)

    reps = c // D
    wb = [
        _block_diag(np.asarray(W, np.float32), reps) for W in (W0, W1, W2)
    ]
    eye = np.eye(c, dtype=np.float32)
    w_all = np.stack(
        [wb[0], wb[1], wb[2], eye,
         SCALE * wb[1], SCALE * wb[2], SCALE * eye]
    ).astype(np.float16)

    xf = np.asarray(x, np.float32)
    in_maps = []
    for core in range(NCORES):
        xs = xf[core * b_core:(core + 1) * b_core]      # (b_core, n, D)
        xt = np.ascontiguousarray(
            xs.transpose(0, 2, 1).reshape(c, n)
        ).astype(np.float16)
        in_maps.append({"xt": xt, "at": at8, "wt": w_all})
    return in_maps


def gather_output(results, n=N_FULL, b_full=B_FULL):
    b_core = b_full // NCORES
    out = np.empty((b_full, n, D), dtype=np.float32)
    for core in range(NCORES):
        oc = np.asarray(results[core]["out"], np.float32).reshape(b_core, D, n)
        out[core * b_core:(core + 1) * b_core] = oc.transpose(0, 2, 1)
    return out


def run(x, adj, Identity, W0, W1, W2, n=N_FULL, free=512, trace=False,
        use_double_row=True, use_lrelu_act=False, **_ignored):
    from concourse.bass_utils import run_bass_kernel_spmd

    nc = _get_nc(n, free, use_double_row, use_lrelu_act)
    in_maps = prepare_inputs(x, adj, Identity, W0, W1, W2, n)
    core_ids = list(range(NCORES))
    res = run_bass_kernel_spmd(nc, in_maps, core_ids, trace=trace)
    out = gather_output(res.results, n, x.shape[0])
    return out, res


def kernel(x, adj, Identity, W0, W1, W2):
    out, _ = run(x, adj, Identity, W0, W1, W2)
    return out


# revision 24
# speedup vs baseline: 1.1383x; 1.0090x over previous
"""GCN block (3 layers) on 8 trn2 NeuronCores, data-parallel over batch.

Math: each layer is X' = (adj + I) @ lrelu(X @ W).
Fold each layer's weight into the previous layer's output (A(HW) = (AH)W)
so every layer is one adjacency matmul plus an identity add:

    H0 = lrelu(X0 W0)
    layer l:  G_l = H_l W_{l+1}   (W3 := I)
              Z   = adj @ G_l + G_l
              H_{l+1} = lrelu(Z)   (no lrelu after layer 2)

Key precision/bandwidth trick: adj entries are uniform in [0, 2/N], tiny
relative to the identity term, so the adjacency product tolerates fp8.
We store  at8 = fp8_e4m3(S * adj^T)  with S=2048 (entries land in [0,1])
— 16 MB — which fits ENTIRELY in SBUF (128 KB/partition of 208), so it
is streamed from HBM exactly once (vs 3x for fp16 in the old version),
and fp8 runs the PE at 2x bf16 rate via DoubleRow perf mode (256-deep
contraction per instruction).

The identity term must not see fp8 noise: it is accumulated into the
same PSUM bank by one extra fp16 matmul with S*W_blk stationary, so
PSUM holds S*(adj@G + G) and the descale folds into the lrelu constants.

Per core: 8 samples x 16 features = 128 = partition width. Layouts:
    T-layout  [c=(b,d), m]   (128 partitions, N free)  for H
    N-layout  [m(part), mt, c]                          for G (fp8)
Layer 0 streams A^T panels (m-outer, all 8 output chunks accumulate in
parallel across all 8 PSUM banks). Layers 1-2 run chunk-major from the
resident A^T so each chunk's lrelu + next-layer G tiles overlap the
following chunk's accumulation.
"""

import numpy as np

N_FULL = 4096
D = 16
B_FULL = 64
NCORES = 8
B_CORE = B_FULL // NCORES  # 8
C = B_CORE * D  # 128 partitions
P = 128
NEG_SLOPE = 0.2
SCALE = 2048.0

_CACHE = {}


def _build_nc(n, free, use_double_row=True, use_lrelu_act=False):
    import concourse.bass as bass
    import concourse.mybir as mybir
    import concourse.tile as tile
    from concourse import bacc

    f32 = mybir.dt.float32
    f16 = mybir.dt.float16
    f8 = mybir.dt.float8e4
    u8 = mybir.dt.uint8
    AF = mybir.ActivationFunctionType
    ALU = mybir.AluOpType
    DR = mybir.MatmulPerfMode.DoubleRow if use_double_row else None

    nt = n // P          # 32 m-tiles
    nch = n // free      # 8 output column chunks
    tpb = 2              # m-tiles per A^T panel (= DoubleRow k-pair)
    mb = n // (tpb * P)  # 16 panels

    nc = bacc.Bacc(
        "TRN2", target_bir_lowering=False, debug=False, num_devices=NCORES
    )
    xt_h = nc.dram_tensor("xt", [C, n], f16, kind="ExternalInput")
    at_h = nc.dram_tensor("at", [n, n], u8, kind="ExternalInput")
    w_h = nc.dram_tensor("wt", [7, P, P], f16, kind="ExternalInput")
    out_h = nc.dram_tensor("out", [C, n], f16, kind="ExternalOutput")

    def panel_src(i):
        return at_h[i * tpb * P:(i + 1) * tpb * P, :].rearrange(
            "(t p) n -> p t n", p=P
        )

    with tile.TileContext(nc) as tc:
        with (
            tc.tile_pool(name="const", bufs=1) as constp,
            tc.tile_pool(name="htp", bufs=2) as htp,
            tc.tile_pool(name="g8p", bufs=2) as g8p,
            tc.tile_pool(name="outp", bufs=4) as outp,
            tc.tile_pool(name="lkp", bufs=4) as lkp,
            tc.tile_pool(name="psp", bufs=8, space="PSUM") as psp,
        ):
            # One DMA queue (Sync), strict priority order: X^T halves
            # (prepass input) first, then weights, then the 16 A^T
            # panels back-to-back. Splitting across queues does NOT add
            # bandwidth on trn2 (the per-core HBM port is the cap) and
            # only starves the latency-critical transfers.
            # Weight slots: 0: S*W0 (prepass, makes PSUM = S*(X W0) so
            # H is carried as S*H in fp16); 1-3: W1/S, W2/S, I/S (tiny
            # path, (S*H) @ (W/S) = G); 4-6: W1, W2, I (identity path,
            # (S*H) @ W = S*G).
            xt_sb = constp.tile([C, n], f16)
            h = n // 2
            nc.sync.dma_start(xt_sb[:, :h], xt_h[:, :h])
            nc.sync.dma_start(xt_sb[:, h:], xt_h[:, h:])
            w_sb = constp.tile([P, 7, P], f16)
            nc.sync.dma_start(w_sb[:], w_h[:].rearrange("w p q -> p w q"))

            at_res = [
                constp.tile([P, tpb, n], u8, name=f"atc{i}") for i in range(mb)
            ]
            for i in range(mb):
                nc.sync.dma_start(at_res[i][:], panel_src(i))

            def at_mm(i):  # fp8 view of a resident panel
                return at_res[i].bitcast(f8)

            def lrelu(dest, ps):
                # H is kept pre-scaled by S in fp16 (the descale folds
                # into the weight slots), so lrelu is one DVE op:
                #   dest = max(NEG*ps, ps) = S*lrelu(Z)   (ps = S*Z)
                if use_lrelu_act:
                    nc.vector.scalar_tensor_tensor(
                        dest, ps[:], NEG_SLOPE, ps[:], ALU.mult, ALU.max
                    )
                    return
                # fallback (one extra ACT op, no dual PSUM read): ACT
                # copies ps to SBUF, DVE does max(NEG*t, t) from SBUF.
                t = lkp.tile([P, dest.shape[-1]], f16, tag="lk", name="lk")
                nc.scalar.copy(t[:], ps[:])
                nc.vector.scalar_tensor_tensor(
                    dest, t[:], NEG_SLOPE, t[:], ALU.mult, ALU.max
                )

            # ---- prepass: H0^T = lrelu(W0_blk.T @ X^T) (T-layout) ----
            ht_cur = htp.tile([C, n], f16, tag="ht", name="ht0")
            for ch in range(nch):
                sl = slice(ch * free, (ch + 1) * free)
                ps = psp.tile([P, free], f32, tag="ps", name="psx")
                nc.tensor.matmul(
                    ps[:], w_sb[:, 0, :], xt_sb[:, sl], start=True, stop=True
                )
                lrelu(ht_cur[:, sl], ps)

            tpc = nt // nch  # m-tiles per output chunk

            def make_g8(ht, w_idx, ncx, g8_dst):
                # G tiles (N-layout, fp8) for chunk ncx's m-tiles: tpc
                # tiny transpose-matmuls into slices of ONE psum bank,
                # then a single wide cast-copy (alternating DVE/ACT).
                pst = psp.tile([P, tpc, P], f32, tag="ps", name="pst")
                for j in range(tpc):
                    mt = ncx * tpc + j
                    nc.tensor.matmul(
                        pst[:, j, :], ht[:, mt * P:(mt + 1) * P],
                        w_sb[:, w_idx, :], start=True, stop=True,
                    )
                dst = g8_dst[:, ncx * tpc:(ncx + 1) * tpc, :]
                if ncx % 2 == 0:
                    nc.vector.tensor_copy(dst, pst[:])
                else:
                    nc.scalar.copy(dst, pst[:])

            # ---- layer 0: G0 fp8, then m-outer streamed big matmul ----
            g8_cur = g8p.tile([P, nt, P], f8, tag="g8", name="g80")
            for ncx in range(nch):
                make_g8(ht_cur, 1, ncx, g8_cur)

            ps_list = [
                psp.tile([P, free], f32, tag="ps", name=f"ps0c{i}")
                for i in range(nch)
            ]
            for ncx in range(nch):
                sl = slice(ncx * free, (ncx + 1) * free)
                nc.tensor.matmul(
                    ps_list[ncx][:], w_sb[:, 4, :], ht_cur[:, sl],
                    start=True, stop=False,
                )
            for mbx in range(mb):
                for ncx in range(nch):
                    sl = slice(ncx * free, (ncx + 1) * free)
                    nc.tensor.matmul(
                        ps_list[ncx][:],
                        g8_cur[:, tpb * mbx:tpb * (mbx + 1), :],
                        at_mm(mbx)[:, :, sl],
                        perf_mode=DR,
                        start=False,
                        stop=(mbx == mb - 1),
                    )

            # layer 0 -> 1 turnaround: lrelu each chunk, then build G1
            # tiles for that chunk so PE/ACT/DVE pipeline across chunks.
            ht_nxt = htp.tile([C, n], f16, tag="ht", name="ht1")
            g8_nxt = g8p.tile([P, nt, P], f8, tag="g8", name="g81")
            for ncx in range(nch):
                sl = slice(ncx * free, (ncx + 1) * free)
                lrelu(ht_nxt[:, sl], ps_list[ncx])
                make_g8(ht_nxt, 2, ncx, g8_nxt)
            ht_cur, g8_cur = ht_nxt, g8_nxt

            # ---- layers 1-2: chunk-major from resident A^T ----
            for layer in (1, 2):
                last = layer == 2
                if not last:
                    ht_nxt = htp.tile([C, n], f16, tag="ht", name="ht2")
                    g8_nxt = g8p.tile([P, nt, P], f8, tag="g8", name="g82")
                ps_l = [None] * nch

                def issue(ncx, layer=layer):
                    sl = slice(ncx * free, (ncx + 1) * free)
                    ps = psp.tile([P, free], f32, tag="ps", name=f"psL{layer}")
                    nc.tensor.matmul(
                        ps[:], w_sb[:, 4 + layer, :], ht_cur[:, sl],
                        start=True, stop=False,
                    )
                    for kt in range(mb):
                        nc.tensor.matmul(
                            ps[:],
                            g8_cur[:, tpb * kt:tpb * (kt + 1), :],
                            at_mm(kt)[:, :, sl],
                            perf_mode=DR,
                            start=False,
                            stop=(kt == mb - 1),
                        )
                    return ps

                def finish(ncx, last=last, ht_nxt=ht_nxt if not last else None,
                           g8_nxt=g8_nxt if not last else None):
                    sl = slice(ncx * free, (ncx + 1) * free)
                    if last:
                        oc = outp.tile([P, free], f16, tag="oc", name="oc")
                        if ncx % 2 == 0:
                            nc.vector.tensor_scalar_mul(
                                oc[:], ps_l[ncx][:], 1.0 / SCALE
                            )
                        else:
                            nc.scalar.mul(oc[:], ps_l[ncx][:], 1.0 / SCALE)
                        nc.sync.dma_start(out_h[:, sl], oc[:])
                    else:
                        lrelu(ht_nxt[:, sl], ps_l[ncx])
                        make_g8(ht_nxt, 3, ncx, g8_nxt)

                # software pipeline: chunk ncx's finish work (lrelu +
                # next-layer G tiles) is issued two chunks later so the
                # ACT+DVE lrelu chain has a full chunk of PE time to
                # complete before the PE reaches the G-tile matmuls.
                lag = 2 if not last else 1
                for ncx in range(nch):
                    ps_l[ncx] = issue(ncx)
                    if ncx >= lag:
                        finish(ncx - lag)
                for ncx in range(nch - lag, nch):
                    finish(ncx)
                if not last:
                    ht_cur, g8_cur = ht_nxt, g8_nxt

    nc.compile()
    return nc


def _get_nc(n, free, use_double_row=True, use_lrelu_act=False):
    key = (n, free, use_double_row, use_lrelu_act)
    if key not in _CACHE:
        _CACHE[key] = _build_nc(n, free, use_double_row, use_lrelu_act)
    return _CACHE[key]


def _block_diag(w, reps):
    d = w.shape[0]
    out = np.zeros((reps * d, reps * d), dtype=np.float32)
    for b in range(reps):
        out[b * d:(b + 1) * d, b * d:(b + 1) * d] = w
    return out


def prepare_inputs(x, adj, Identity, W0, W1, W2, n=N_FULL):
    """Host-side layout prep. Returns per-core input maps."""
    import ml_dtypes

    b_full = x.shape[0]
    b_core = b_full // NCORES
    c = b_core * D

    at8 = (
        np.ascontiguousarray(adj.T.astype(np.float32)) * SCALE
    ).astype(ml_dtypes.float8_e4m3).view(np.uint8)

    reps = c // D
    wb = [
        _block_diag(np.asarray(W, np.float32), reps) for W in (W0, W1, W2)
    ]
    eye = np.eye(c, dtype=np.float32)
    w_all = np.stack(
        [SCALE * wb[0], wb[1] / SCALE, wb[2] / SCALE, eye / SCALE,
         wb[1], wb[2], eye]
    ).astype(np.float16)

    xf = np.asarray(x, np.float32)
    in_maps = []
    for core in range(NCORES):
        xs = xf[core * b_core:(core + 1) * b_core]      # (b_core, n, D)
        xt = np.ascontiguousarray(
            xs.transpose(0, 2, 1).reshape(c, n)
        ).astype(np.float16)
        in_maps.append({"xt": xt, "at": at8, "wt": w_all})
    return in_maps


def gather_output(results, n=N_FULL, b_full=B_FULL):
    b_core = b_full // NCORES
    out = np.empty((b_full, n, D), dtype=np.float32)
    for core in range(NCORES):
        oc = np.asarray(results[core]["out"], np.float32).reshape(b_core, D, n)
        out[core * b_core:(core + 1) * b_core] = oc.transpose(0, 2, 1)
    return out


def run(x, adj, Identity, W0, W1, W2, n=N_FULL, free=512, trace=False,
        use_double_row=True, use_lrelu_act=False, **_ignored):
    from concourse.bass_utils import run_bass_kernel_spmd

    nc = _get_nc(n, free, use_double_row, use_lrelu_act)
    in_maps = prepare_inputs(x, adj, Identity, W0, W1, W2, n)
    core_ids = list(range(NCORES))
    res = run_bass_kernel_spmd(nc, in_maps, core_ids, trace=trace)
    out = gather_output(res.results, n, x.shape[0])
    return out, res


def kernel(x, adj, Identity, W0, W1, W2):
    out, _ = run(x, adj, Identity, W0, W1, W2)
    return out
